# revision 2
# baseline (speedup 1.0000x reference)
"""Fused single-head attention (QKV projection + softmax(QK^T)V) on 8 trn2 cores.

Problem (hardcoded): x [4, 4096, 768] f32, W_qkv [768, 2304] f32, b_qkv [2304] f32.
  qkv = x @ W_qkv + b_qkv ; q,k,v = split(qkv, 3)
  out = softmax(q k^T / sqrt(768)) v          -> [4, 4096, 768] f32

Sharding: batch (4) x key-halves (2) -> 8 cores, no cross-core traffic.
Each core gets one batch's x (pre-transposed on host to xT [768, 4096] fp16,
with the key half it owns rotated into columns [0, 2048)), projects q for
all 4096 queries but k/v only for its 2048 keys, and computes PARTIAL
attention sums over those keys. The host combines pair partials:
(A pair-AllGather q dedup was tried: the ~94us collective cannot hide
behind the ~61us of k/v projection - net loss, reverted.)
out = ((o0 + o1) / (d0 + d1)) / SV + bv.  (v's bias commutes out of the
softmax average because the weights sum to 1, so it is applied on the host.)

Mixed precision (rel-err budget 2e-2, measured ~1.8e-2):
  - projection matmuls fp16 (fp8 here costs ~3.5e-2 alone - too noisy).
  - scores q.k^T: fp8e4m3 DoubleRow (2x PE throughput). q8/k8 are written
    by the projection-evacuation activation directly into the DoubleRow
    interleaved layout [128, 2, n] with h = 256g + 128i + p, scaled by
    SQK=16 (exp scale folds 1/SQK^2 back out).
  - PV: the first F2JT=8 j-tiles (half the keys) run fp8 DoubleRow
    (p8 = 8p e4m3, v8 = 16v e4m3), the rest fp16. Both paths are scaled
    p~=8p, v~=16v so they accumulate consistently in the fp32 PSUM.
  - outputs outT/den fp16 (halves output DMA).

Structure notes:
  - warm-up matmuls cover the HAM clock-gate ramp + initial DMA.
  - x r-block DMAs are staggered between the W column-block DMAs so the
    projection never starves.
  - each i-block's PV for the last j-tile runs ht-by-ht with its PSUM
    evacuation + output DMA chasing each matmul (epilogue pattern), so the
    6 accumulator banks free progressively instead of all-at-once.
PSUM budget (8 banks): scores 2 + out accumulators 6.
"""

import math
from contextlib import ExitStack
from functools import lru_cache

import numpy as np

import concourse.bacc as bacc
import concourse.bass as bass
import concourse.tile as tile
from concourse import mybir
from concourse.bass_utils import run_bass_kernel_spmd

B, N, C = 4, 4096, 768
H = 768          # head dim (== C)
H3 = 3 * H
NCORES = 8
NK = N // 2      # keys per core
DT = mybir.dt.float16
FP8 = mybir.dt.float8e4
F32 = mybir.dt.float32
SCALE = 1.0 / math.sqrt(H)
SQK = 16.0       # fp8 scale for q and k (q8 = SQK*(q+bq))
SV = 16.0        # scale for v (both fp8 and fp16 paths)
SP = 8.0         # scale for p = exp(score) (both paths); exp bias = ln(SP)
DR = mybir.MatmulPerfMode.DoubleRow

CT = C // 128    # 6 contraction tiles (c)
HT = H // 128    # 6 head tiles (h)
HG = HT // 2     # 3 DoubleRow groups of 256 head dims
JT = NK // 128   # 16 key tiles (j) per core
F2JT = 4         # first F2JT j-tiles use fp8 PV (error/speed knob;
                 # 8 measured 2.11e-2 rel err on hw - over the 2e-2 gate;
                 # 4 measures 1.75e-2)
RB = 8           # r-blocks of 512 over the 4096 rows
RBS = N // RB    # 512
KRB = RB // 2    # r-blocks that contain this core's keys (first 4)
IB = 8           # i-blocks of 512 over all 4096 queries
IBS = N // IB    # 512


def build_program():
    nc = bacc.Bacc(
        "TRN2",
        target_bir_lowering=False,
        debug=False,
        enable_asserts=False,
        num_devices=NCORES,
    )
    xT_d = nc.dram_tensor("xT", [C, N], DT, kind="ExternalInput").ap()
    w_d = nc.dram_tensor("w", [C, H3], DT, kind="ExternalInput").ap()
    bqk_d = nc.dram_tensor("bqk", [128, 2 * HT], F32, kind="ExternalInput").ap()
    outT_d = nc.dram_tensor("outT", [H, N], DT, kind="ExternalOutput").ap()
    # per-partition partial softmax denominators; host sums over axis 1
    den_d = nc.dram_tensor("den", [IB, 128, IBS], DT, kind="ExternalOutput").ap()

    with tile.TileContext(nc) as tc:
        with ExitStack() as ctx:
            persist = ctx.enter_context(tc.tile_pool(name="persist", bufs=1))

            # fp8 q/k in DoubleRow-interleaved layout: tile g holds head dims
            # [256g, 256g+256) as [128 partitions(p), 2(i), cols] with
            # h = 256g + 128i + p.
            kT2 = [persist.tile([128, 2, NK], FP8, tag=f"kT2{t}", name=f"kT2{t}")
                   for t in range(HG)]
            qT2 = [persist.tile([128, 2, N], FP8, tag=f"qT2{t}", name=f"qT2{t}")
                   for t in range(HG)]
            # v: fp8 DoubleRow tiles for j-tiles < F2JT (key j = 256gj+128i+p),
            # fp16 tiles for the rest
            v2 = [persist.tile([128, 2, H], FP8, tag=f"v2{t}", name=f"v2{t}")
                  for t in range(F2JT // 2)]
            vv = {jt: persist.tile([128, H], DT, tag=f"v{jt}", name=f"v{jt}")
                  for jt in range(F2JT, JT)}
            bqk = persist.tile([128, 2 * HT], F32, tag="bqk")

            # ---- Phase 1: QKV projection ----
            with tc.tile_pool(name="wpool", bufs=1) as wpool, \
                 tc.tile_pool(name="xpool", bufs=1) as xpool, \
                 tc.tile_pool(name="pj", bufs=4, space="PSUM") as pj, \
                 tc.tile_pool(name="pv", bufs=2, space="PSUM") as pv:

                ws = [wpool.tile([128, H3], DT, tag=f"w{t}", name=f"w{t}")
                      for t in range(CT)]
                # x lives in SBUF for all of phase 1 as 6 full strips; few
                # big DMAs on the otherwise-idle Pool queue (the SP queue's
                # ~650ns per-DMA issue cost serializes small-DMA plans).
                xf = [xpool.tile([128, N], DT, tag=f"xf{t}", name=f"xf{t}")
                      for t in range(CT)]

                # r-block-0 x pieces first, split across the idle Pool and
                # Act DMA queues so they land within the warm-up window.
                for ct in range(CT):
                    q = nc.gpsimd if ct % 2 == 0 else nc.scalar
                    q.dma_start(out=xf[ct][:, 0:RBS],
                                in_=xT_d[ct * 128:(ct + 1) * 128, 0:RBS])
                # W: q columns first (projected before k/v so the q exchange
                # can overlap the k/v projection), then k, then v.
                for ct in range(CT):
                    nc.sync.dma_start(out=ws[ct][:, 0:H],
                                      in_=w_d[ct * 128:(ct + 1) * 128, 0:H])
                nc.sync.dma_start(out=bqk, in_=bqk_d)
                for ct in range(CT):
                    nc.gpsimd.dma_start(
                        out=xf[ct][:, RBS:2 * RBS],
                        in_=xT_d[ct * 128:(ct + 1) * 128, RBS:2 * RBS])
                for ct in range(CT):
                    nc.sync.dma_start(out=ws[ct][:, H:2 * H],
                                      in_=w_d[ct * 128:(ct + 1) * 128, H:2 * H])
                for ct in range(CT):
                    nc.gpsimd.dma_start(
                        out=xf[ct][:, 2 * RBS:NK],
                        in_=xT_d[ct * 128:(ct + 1) * 128, 2 * RBS:NK])
                for ct in range(CT):
                    nc.gpsimd.dma_start(
                        out=xf[ct][:, NK:N],
                        in_=xT_d[ct * 128:(ct + 1) * 128, NK:N])
                for ct in range(CT):
                    nc.sync.dma_start(out=ws[ct][:, 2 * H:H3],
                                      in_=w_d[ct * 128:(ct + 1) * 128, 2 * H:H3])

                # PE warm-up: ~4.3us of junk matmuls (no DMA deps) so the
                # HAM clock-gate reaches full rate while the first x/W DMAs
                # are still in flight.
                warm_l = wpool.tile([128, 128], DT, tag="warml", name="warml")
                nc.vector.memset(warm_l, 0.0)
                for i in range(40):
                    wp = pj.tile([128, RBS], F32, tag="pj", name=f"warm{i}")
                    nc.tensor.matmul(wp[:, 0:128], warm_l, warm_l,
                                     start=True, stop=True)

                # q projection for all 4096 queries (duplicated per pair:
                # a pair-AllGather dedup was tried and lost ~13us - the
                # collective costs ~94us and only ~61us of k/v hides it)
                for rb in range(RB):
                    r0 = rb * RBS
                    for ht in range(HT):
                        ps = pj.tile([128, RBS], F32, tag="pj")
                        for ct in range(CT):
                            nc.tensor.matmul(
                                ps,
                                ws[ct][:, ht * 128:(ht + 1) * 128],
                                xf[ct][:, r0:r0 + RBS],
                                start=(ct == 0), stop=(ct == CT - 1),
                            )
                        # q8 = fp8(SQK*(q + b)) into interleaved slice
                        # (g, i) = (ht//2, ht%2); host pre-scales bias.
                        nc.scalar.activation(
                            out=qT2[ht // 2][:, ht % 2, r0:r0 + RBS],
                            in_=ps,
                            func=mybir.ActivationFunctionType.Identity,
                            scale=SQK,
                            bias=bqk[:, ht:ht + 1],
                        )

                # k + v projection
                for rb in range(KRB):
                    r0 = rb * RBS
                    for ht in range(HT):
                        ps = pj.tile([128, RBS], F32, tag="pj")
                        for ct in range(CT):
                            nc.tensor.matmul(
                                ps,
                                ws[ct][:, H + ht * 128: H + (ht + 1) * 128],
                                xf[ct][:, r0:r0 + RBS],
                                start=(ct == 0), stop=(ct == CT - 1),
                            )
                        nc.scalar.activation(
                            out=kT2[ht // 2][:, ht % 2, r0:r0 + RBS],
                            in_=ps,
                            func=mybir.ActivationFunctionType.Identity,
                            scale=SQK,
                            bias=bqk[:, HT + ht:HT + ht + 1],
                        )

                    if True:
                        for j in range(RBS // 128):
                            jt = rb * (RBS // 128) + j
                            ps = pv.tile([128, H], F32, tag="pv")
                            for ct in range(CT):
                                xs = xf[ct][:, r0 + j * 128:r0 + (j + 1) * 128]
                                nc.tensor.matmul(
                                    ps[:, 0:512], xs, ws[ct][:, 2 * H: 2 * H + 512],
                                    start=(ct == 0), stop=(ct == CT - 1))
                                nc.tensor.matmul(
                                    ps[:, 512:H], xs, ws[ct][:, 2 * H + 512: 3 * H],
                                    start=(ct == 0), stop=(ct == CT - 1))
                            # v~ = SV*v (bias applied on host); ScalarE scale
                            # immediate (DVE tensor_scalar lowers to a slow
                            # TensorScalarPtr)
                            vdst = (v2[jt // 2][:, jt % 2, :] if jt < F2JT
                                    else vv[jt])
                            nc.scalar.activation(
                                out=vdst, in_=ps,
                                func=mybir.ActivationFunctionType.Identity,
                                scale=SV)

            # ---- Phase 2: attention (partial sums over this core's keys) ----
            with tc.tile_pool(name="ppool", bufs=1) as ppool, \
                 tc.tile_pool(name="opool", bufs=8) as opool, \
                 tc.tile_pool(name="spool", bufs=2) as spool, \
                 tc.tile_pool(name="ps_s", bufs=2, space="PSUM") as ps_s, \
                 tc.tile_pool(name="ps_o", bufs=6, space="PSUM") as ps_o:
                p2 = [ppool.tile([128, 2, IBS], FP8, tag=f"p2{t}", name=f"p2{t}")
                      for t in range(F2JT // 2)]
                p_t = {jt: ppool.tile([128, IBS], DT, tag=f"p{jt}", name=f"p{jt}")
                       for jt in range(F2JT, JT)}
                lnsp = ppool.tile([128, 1], F32, tag="lnsp", name="lnsp")
                nc.vector.memset(lnsp, math.log(SP))

                pending = []   # deferred work, flushed between PE groups

                def flush():
                    while pending:
                        pending.pop(0)()

                def emit_pv8(og, gj, start=False):
                    def go():
                        for ht in range(HT):
                            nc.tensor.matmul(
                                og[ht],
                                v2[gj][:, :, ht * 128:(ht + 1) * 128],
                                p2[gj],
                                start=start, stop=False,
                                perf_mode=DR,
                            )
                    pending.append(go)

                def emit_pv16(og, jt, start=False):
                    def go():
                        for ht in range(HT):
                            nc.tensor.matmul(
                                og[ht],
                                vv[jt][:, ht * 128:(ht + 1) * 128],
                                p_t[jt],
                                start=start, stop=False,
                            )
                    pending.append(go)

                # j-tile processing order: fp16 tiles first (their PV chases a
                # single exp, filling the pipeline early), then the fp8 pairs;
                # the final pair (gj = F2JT//2-1) finishes ht-by-ht with its
                # PSUM evacuation + output DMA chasing each matmul.
                jts_order = list(range(F2JT, JT)) + list(range(F2JT))
                GJ_LAST = F2JT // 2 - 1
                for ib in range(IB):
                    i0 = ib * IBS
                    og = [ps_o.tile([128, IBS], F32, tag="o", name=f"o{ib}_{g}")
                          for g in range(HT)]
                    Sf = spool.tile([128, IBS], F32, tag="Sf", name=f"Sf{ib}")
                    for pos, jt in enumerate(jts_order):
                        sps = ps_s.tile([128, IBS], F32, tag="s")
                        for g in range(HG):
                            nc.tensor.matmul(
                                sps,
                                kT2[g][:, :, jt * 128:(jt + 1) * 128],
                                qT2[g][:, :, i0:i0 + IBS],
                                start=(g == 0), stop=(g == HG - 1),
                                perf_mode=DR,
                            )
                        flush()
                        # p~ = SP * exp(score): bias ln(SP) inside the exp
                        pdst = (p2[jt // 2][:, jt % 2, :] if jt < F2JT
                                else p_t[jt])
                        nc.scalar.activation(
                            out=pdst, in_=sps,
                            func=mybir.ActivationFunctionType.Exp,
                            scale=SCALE / (SQK * SQK),
                            bias=lnsp[:, 0:1],
                        )
                        if pos == 0:
                            nc.vector.tensor_copy(out=Sf, in_=pdst)
                        else:
                            nc.vector.tensor_add(Sf, Sf, pdst)
                        # PV runs behind the scores pipeline (last i-block:
                        # PV is done ht-major below instead)
                        if ib < IB - 1:
                            if jt >= F2JT:
                                emit_pv16(og, jt, start=(pos == 0))
                            elif jt % 2 == 1 and jt // 2 != GJ_LAST:
                                emit_pv8(og, jt // 2)
                    S16 = spool.tile([128, IBS], DT, tag="S16", name=f"S16{ib}")
                    nc.vector.tensor_copy(out=S16, in_=Sf)

                    if ib < IB - 1:
                        def finish(og=og, S16=S16, ib=ib, i0=i0):
                            nc.sync.dma_start(out=den_d[ib], in_=S16)
                            for ht in range(HT):
                                nc.tensor.matmul(
                                    og[ht],
                                    v2[GJ_LAST][:, :, ht * 128:(ht + 1) * 128],
                                    p2[GJ_LAST],
                                    start=False, stop=True,
                                    perf_mode=DR,
                                )
                                ot = opool.tile([128, IBS], DT, tag="ot",
                                                name=f"ot{i0}_{ht}")
                                # keep the Act queue free for the next
                                # i-block's exps -> evacuate on DVE only
                                nc.vector.tensor_copy(out=ot, in_=og[ht])
                                dma = (nc.scalar.dma_start if ht % 2
                                       else nc.sync.dma_start)
                                dma(out=outT_d[ht * 128:(ht + 1) * 128,
                                               i0:i0 + IBS],
                                    in_=ot)
                        pending.append(finish)
                    else:
                        # last i-block: ht-major PV so each accumulator is
                        # complete (and its output DMA in flight) as early as
                        # possible -- shrinks the end-of-kernel tail.
                        flush()
                        nc.sync.dma_start(out=den_d[ib], in_=S16)
                        for ht in range(HT):
                            for pos2, jt in enumerate(jts_order):
                                if jt < F2JT:
                                    if jt % 2 == 1:
                                        continue
                                    nc.tensor.matmul(
                                        og[ht],
                                        v2[jt // 2][:, :,
                                                    ht * 128:(ht + 1) * 128],
                                        p2[jt // 2],
                                        start=False,
                                        stop=(jt // 2 == GJ_LAST),
                                        perf_mode=DR,
                                    )
                                else:
                                    nc.tensor.matmul(
                                        og[ht],
                                        vv[jt][:, ht * 128:(ht + 1) * 128],
                                        p_t[jt],
                                        start=(pos2 == 0), stop=False,
                                    )
                            ot = opool.tile([128, IBS], DT, tag="ot",
                                            name=f"ot{i0}_{ht}")
                            if ht % 2 == 0:
                                nc.vector.tensor_copy(out=ot, in_=og[ht])
                                dma = nc.sync.dma_start
                            else:
                                nc.scalar.activation(
                                    out=ot, in_=og[ht],
                                    func=mybir.ActivationFunctionType.Copy)
                                dma = nc.scalar.dma_start
                            dma(out=outT_d[ht * 128:(ht + 1) * 128,
                                           i0:i0 + IBS],
                                in_=ot)
                flush()
    nc.compile()
    return nc


@lru_cache(maxsize=1)
def _cached_program():
    return build_program()


def _prep_in_maps(x, W_qkv, b_qkv):
    x = np.asarray(x, dtype=np.float32)
    W_qkv = np.asarray(W_qkv, dtype=np.float32)
    b_qkv = np.asarray(b_qkv, dtype=np.float32)
    w16 = W_qkv.astype(np.float16)
    bq = b_qkv[0:H].astype(np.float32).reshape(HT, 128).T    # [128, HT]
    bk = b_qkv[H:2 * H].astype(np.float32).reshape(HT, 128).T
    # activation computes fp8(SQK*psum + bias) -> bias must carry SQK
    bqk = np.ascontiguousarray(
        SQK * np.concatenate([bq, bk], axis=1))  # [128, 2*HT]

    in_maps = []
    for core in range(NCORES):
        b, kh = core // 2, core % 2
        xb = x[b]  # [N, C] f32
        if kh == 1:
            # Rotate so this core's key rows occupy rows [0, NK). Queries are
            # also rotated; the host rotates this core's outputs back.
            xb = np.concatenate([xb[NK:], xb[:NK]], axis=0)
        xT = np.ascontiguousarray(xb.T).astype(np.float16)
        in_maps.append({"xT": xT, "w": w16, "bqk": bqk})
    return in_maps


def _combine(results, b_qkv):
    bv = np.asarray(b_qkv, dtype=np.float32)[2 * H:3 * H]
    out = np.empty((B, N, C), dtype=np.float32)
    for b in range(B):
        o0 = results[2 * b]["outT"].astype(np.float32)   # [H, N]
        d0 = results[2 * b]["den"].astype(np.float32).sum(axis=1).reshape(N)
        o1 = results[2 * b + 1]["outT"].astype(np.float32)
        d1 = results[2 * b + 1]["den"].astype(np.float32).sum(axis=1).reshape(N)
        # core (2b+1) worked in rotated query order; rotate back
        o1 = np.concatenate([o1[:, NK:], o1[:, :NK]], axis=1)
        d1 = np.concatenate([d1[NK:], d1[:NK]])
        # o is (SP*SV)-scaled, den SP-scaled; v bias commutes out of softmax
        out[b] = (((o0 + o1) / (d0 + d1)) / SV).T + bv
    return out


def kernel(x, W_qkv, b_qkv):
    nc = _cached_program()
    in_maps = _prep_in_maps(x, W_qkv, b_qkv)
    res = run_bass_kernel_spmd(nc, in_maps, core_ids=list(range(NCORES)))
    return _combine(res.results, b_qkv)


# revision 4
# speedup vs baseline: 1.0021x; 1.0021x over previous
"""Fused single-head attention (QKV projection + softmax(QK^T)V) on 8 trn2 cores.

Problem (hardcoded): x [4, 4096, 768] f32, W_qkv [768, 2304] f32, b_qkv [2304] f32.
  qkv = x @ W_qkv + b_qkv ; q,k,v = split(qkv, 3)
  out = softmax(q k^T / sqrt(768)) v          -> [4, 4096, 768] f32

Sharding: batch (4) x key-halves (2) -> 8 cores, no cross-core traffic.
Each core gets one batch's x (pre-transposed on host to xT [768, 4096] fp16,
with the key half it owns rotated into columns [0, 2048)), projects q for
all 4096 queries but k/v only for its 2048 keys, and computes PARTIAL
attention sums over those keys. The host combines pair partials
(a pair-AllGather q dedup was tried: the ~94us collective cannot hide
behind the ~61us of k/v projection - net loss, reverted):
out = ((o0 + o1) / (d0 + d1)) / SV + bv.  (v's bias commutes out of the
softmax average because the weights sum to 1, so it is applied on the host.)

Mixed precision (rel-err budget 2e-2, measured ~1.8e-2):
  - projection matmuls fp16 (fp8 here costs ~3.5e-2 alone - too noisy).
  - scores q.k^T: fp8e4m3 DoubleRow (2x PE throughput). q8/k8 are written
    by the projection-evacuation activation directly into the DoubleRow
    interleaved layout [128, 2, n] with h = 256g + 128i + p, scaled by
    SQK=16 (exp scale folds 1/SQK^2 back out).
  - PV: the first F2JT=8 j-tiles (half the keys) run fp8 DoubleRow
    (p8 = 8p e4m3, v8 = 16v e4m3), the rest fp16. Both paths are scaled
    p~=8p, v~=16v so they accumulate consistently in the fp32 PSUM.
  - outputs outT/den fp16 (halves output DMA).

Structure notes:
  - warm-up matmuls cover the HAM clock-gate ramp + initial DMA.
  - x r-block DMAs are staggered between the W column-block DMAs so the
    projection never starves.
  - each i-block's PV for the last j-tile runs ht-by-ht with its PSUM
    evacuation + output DMA chasing each matmul (epilogue pattern), so the
    6 accumulator banks free progressively instead of all-at-once.
PSUM budget (8 banks): scores 2 + out accumulators 6.
"""

import math
from contextlib import ExitStack
from functools import lru_cache

import numpy as np

import concourse.bacc as bacc
import concourse.bass as bass
import concourse.tile as tile
from concourse import mybir
from concourse.bass_utils import run_bass_kernel_spmd

B, N, C = 4, 4096, 768
H = 768          # head dim (== C)
H3 = 3 * H
NCORES = 8
NK = N // 2      # keys per core
DT = mybir.dt.float16
FP8 = mybir.dt.float8e4
F32 = mybir.dt.float32
SCALE = 1.0 / math.sqrt(H)
SQK = 16.0       # fp8 scale for q and k (q8 = SQK*(q+bq))
SV = 16.0        # scale for v (both fp8 and fp16 paths)
SP = 8.0         # scale for p = exp(score) (both paths); exp bias = ln(SP)
DR = mybir.MatmulPerfMode.DoubleRow

CT = C // 128    # 6 contraction tiles (c)
HT = H // 128    # 6 head tiles (h)
HG = HT // 2     # 3 DoubleRow groups of 256 head dims
JT = NK // 128   # 16 key tiles (j) per core
F2JT = 4         # first F2JT j-tiles use fp8 PV (error/speed knob;
                 # 8 measured 2.11e-2 rel err on hw - over the 2e-2 gate;
                 # 4 measures 1.75e-2)
RB = 8           # r-blocks of 512 over the 4096 rows
RBS = N // RB    # 512
KRB = RB // 2    # r-blocks that contain this core's keys (first 4)
IB = 8           # i-blocks of 512 over all 4096 queries
IBS = N // IB    # 512


def build_program():
    nc = bacc.Bacc(
        "TRN2",
        target_bir_lowering=False,
        debug=False,
        enable_asserts=False,
        num_devices=NCORES,
    )
    xT_d = nc.dram_tensor("xT", [C, N], DT, kind="ExternalInput").ap()
    w_d = nc.dram_tensor("w", [C, H3], DT, kind="ExternalInput").ap()
    bqk_d = nc.dram_tensor("bqk", [128, 2 * HT], F32, kind="ExternalInput").ap()
    outT_d = nc.dram_tensor("outT", [H, N], DT, kind="ExternalOutput").ap()
    # per-partition partial softmax denominators; host sums over axis 1
    den_d = nc.dram_tensor("den", [IB, 128, IBS], DT, kind="ExternalOutput").ap()

    with tile.TileContext(nc) as tc:
        with ExitStack() as ctx:
            persist = ctx.enter_context(tc.tile_pool(name="persist", bufs=1))

            # fp8 q/k in DoubleRow-interleaved layout: tile g holds head dims
            # [256g, 256g+256) as [128 partitions(p), 2(i), cols] with
            # h = 256g + 128i + p.
            kT2 = [persist.tile([128, 2, NK], FP8, tag=f"kT2{t}", name=f"kT2{t}")
                   for t in range(HG)]
            qT2 = [persist.tile([128, 2, N], FP8, tag=f"qT2{t}", name=f"qT2{t}")
                   for t in range(HG)]
            # v: fp8 DoubleRow tiles for j-tiles < F2JT (key j = 256gj+128i+p),
            # fp16 tiles for the rest
            v2 = [persist.tile([128, 2, H], FP8, tag=f"v2{t}", name=f"v2{t}")
                  for t in range(F2JT // 2)]
            vv = {jt: persist.tile([128, H], DT, tag=f"v{jt}", name=f"v{jt}")
                  for jt in range(F2JT, JT)}
            bqk = persist.tile([128, 2 * HT], F32, tag="bqk")

            # ---- Phase 1: QKV projection ----
            with tc.tile_pool(name="wpool", bufs=1) as wpool, \
                 tc.tile_pool(name="xpool", bufs=1) as xpool, \
                 tc.tile_pool(name="pj", bufs=4, space="PSUM") as pj, \
                 tc.tile_pool(name="pv", bufs=2, space="PSUM") as pv:

                ws = [wpool.tile([128, H3], DT, tag=f"w{t}", name=f"w{t}")
                      for t in range(CT)]
                # x lives in SBUF for all of phase 1 as 6 full strips; few
                # big DMAs on the otherwise-idle Pool queue (the SP queue's
                # ~650ns per-DMA issue cost serializes small-DMA plans).
                xf = [xpool.tile([128, N], DT, tag=f"xf{t}", name=f"xf{t}")
                      for t in range(CT)]

                # warm-up operand memset goes first on the Pool queue (61ns
                # there) so the warm-up matmuls start immediately
                warm_l = wpool.tile([128, 128], DT, tag="warml", name="warml")
                nc.gpsimd.memset(warm_l, 0.0)
                # r-block-0 x pieces first, split across the idle Pool and
                # Act DMA queues so they land within the warm-up window.
                for ct in range(CT):
                    q = nc.gpsimd if ct % 2 == 0 else nc.scalar
                    q.dma_start(out=xf[ct][:, 0:RBS],
                                in_=xT_d[ct * 128:(ct + 1) * 128, 0:RBS])
                # W: q columns first (projected before k/v), then k, then v.
                for ct in range(CT):
                    nc.sync.dma_start(out=ws[ct][:, 0:H],
                                      in_=w_d[ct * 128:(ct + 1) * 128, 0:H])
                nc.sync.dma_start(out=bqk, in_=bqk_d)
                for ct in range(CT):
                    nc.gpsimd.dma_start(
                        out=xf[ct][:, RBS:2 * RBS],
                        in_=xT_d[ct * 128:(ct + 1) * 128, RBS:2 * RBS])
                for ct in range(CT):
                    nc.sync.dma_start(out=ws[ct][:, H:2 * H],
                                      in_=w_d[ct * 128:(ct + 1) * 128, H:2 * H])
                for ct in range(CT):
                    nc.gpsimd.dma_start(
                        out=xf[ct][:, 2 * RBS:NK],
                        in_=xT_d[ct * 128:(ct + 1) * 128, 2 * RBS:NK])
                for ct in range(CT):
                    nc.gpsimd.dma_start(
                        out=xf[ct][:, NK:N],
                        in_=xT_d[ct * 128:(ct + 1) * 128, NK:N])
                for ct in range(CT):
                    nc.sync.dma_start(out=ws[ct][:, 2 * H:H3],
                                      in_=w_d[ct * 128:(ct + 1) * 128, 2 * H:H3])

                # PE warm-up: ~4.3us of junk matmuls (no DMA deps) so the
                # HAM clock-gate reaches full rate while the first x/W DMAs
                # are still in flight.
                for i in range(40):
                    wp = pj.tile([128, RBS], F32, tag="pj", name=f"warm{i}")
                    nc.tensor.matmul(wp[:, 0:128], warm_l, warm_l,
                                     start=True, stop=True)

                # q projection for all 4096 queries (duplicated per pair:
                # a pair-AllGather dedup was tried and lost ~13us - the
                # collective costs ~94us and only ~61us of k/v hides it)
                for rb in range(RB):
                    r0 = rb * RBS
                    for ht in range(HT):
                        ps = pj.tile([128, RBS], F32, tag="pj")
                        for ct in range(CT):
                            nc.tensor.matmul(
                                ps,
                                ws[ct][:, ht * 128:(ht + 1) * 128],
                                xf[ct][:, r0:r0 + RBS],
                                start=(ct == 0), stop=(ct == CT - 1),
                            )
                        # q8 = fp8(SQK*(q + b)) into interleaved slice
                        # (g, i) = (ht//2, ht%2); host pre-scales bias.
                        nc.scalar.activation(
                            out=qT2[ht // 2][:, ht % 2, r0:r0 + RBS],
                            in_=ps,
                            func=mybir.ActivationFunctionType.Identity,
                            scale=SQK,
                            bias=bqk[:, ht:ht + 1],
                        )

                # k + v projection
                for rb in range(KRB):
                    r0 = rb * RBS
                    for ht in range(HT):
                        ps = pj.tile([128, RBS], F32, tag="pj")
                        for ct in range(CT):
                            nc.tensor.matmul(
                                ps,
                                ws[ct][:, H + ht * 128: H + (ht + 1) * 128],
                                xf[ct][:, r0:r0 + RBS],
                                start=(ct == 0), stop=(ct == CT - 1),
                            )
                        nc.scalar.activation(
                            out=kT2[ht // 2][:, ht % 2, r0:r0 + RBS],
                            in_=ps,
                            func=mybir.ActivationFunctionType.Identity,
                            scale=SQK,
                            bias=bqk[:, HT + ht:HT + ht + 1],
                        )

                    if True:
                        for j in range(RBS // 128):
                            jt = rb * (RBS // 128) + j
                            ps = pv.tile([128, H], F32, tag="pv")
                            for ct in range(CT):
                                xs = xf[ct][:, r0 + j * 128:r0 + (j + 1) * 128]
                                nc.tensor.matmul(
                                    ps[:, 0:512], xs, ws[ct][:, 2 * H: 2 * H + 512],
                                    start=(ct == 0), stop=(ct == CT - 1))
                                nc.tensor.matmul(
                                    ps[:, 512:H], xs, ws[ct][:, 2 * H + 512: 3 * H],
                                    start=(ct == 0), stop=(ct == CT - 1))
                            # v~ = SV*v (bias applied on host); ScalarE scale
                            # immediate (DVE tensor_scalar lowers to a slow
                            # TensorScalarPtr)
                            vdst = (v2[jt // 2][:, jt % 2, :] if jt < F2JT
                                    else vv[jt])
                            nc.scalar.activation(
                                out=vdst, in_=ps,
                                func=mybir.ActivationFunctionType.Identity,
                                scale=SV)

            # ---- Phase 2: attention (partial sums over this core's keys) ----
            with tc.tile_pool(name="ppool", bufs=1) as ppool, \
                 tc.tile_pool(name="opool", bufs=8) as opool, \
                 tc.tile_pool(name="spool", bufs=2) as spool, \
                 tc.tile_pool(name="ps_s", bufs=2, space="PSUM") as ps_s, \
                 tc.tile_pool(name="ps_o", bufs=6, space="PSUM") as ps_o:
                p2 = [ppool.tile([128, 2, IBS], FP8, tag=f"p2{t}", name=f"p2{t}")
                      for t in range(F2JT // 2)]
                p_t = {jt: ppool.tile([128, IBS], DT, tag=f"p{jt}", name=f"p{jt}")
                       for jt in range(F2JT, JT)}
                lnsp = ppool.tile([128, 1], F32, tag="lnsp", name="lnsp")
                nc.vector.memset(lnsp, math.log(SP))

                pending = []   # deferred work, flushed between PE groups

                def flush():
                    while pending:
                        pending.pop(0)()

                def emit_pv8(og, gj, start=False):
                    def go():
                        for ht in range(HT):
                            nc.tensor.matmul(
                                og[ht],
                                v2[gj][:, :, ht * 128:(ht + 1) * 128],
                                p2[gj],
                                start=start, stop=False,
                                perf_mode=DR,
                            )
                    pending.append(go)

                def emit_pv16(og, jt, start=False):
                    def go():
                        for ht in range(HT):
                            nc.tensor.matmul(
                                og[ht],
                                vv[jt][:, ht * 128:(ht + 1) * 128],
                                p_t[jt],
                                start=start, stop=False,
                            )
                    pending.append(go)

                # j-tile processing order: fp16 tiles first (their PV chases a
                # single exp, filling the pipeline early), the fp8 pairs
                # mid-sequence (their PV needs BOTH exps of the pair - burying
                # them among fp16 tiles hides that latency), fp16 again last;
                # the final j-tile finishes ht-by-ht with its PSUM evacuation
                # + output DMA chasing each matmul.
                jts_order = (list(range(F2JT, F2JT + 4)) + list(range(F2JT))
                             + list(range(F2JT + 4, JT)))
                for ib in range(IB):
                    i0 = ib * IBS
                    og = [ps_o.tile([128, IBS], F32, tag="o", name=f"o{ib}_{g}")
                          for g in range(HT)]
                    Sf = spool.tile([128, IBS], F32, tag="Sf", name=f"Sf{ib}")
                    for pos, jt in enumerate(jts_order):
                        sps = ps_s.tile([128, IBS], F32, tag="s")
                        for g in range(HG):
                            nc.tensor.matmul(
                                sps,
                                kT2[g][:, :, jt * 128:(jt + 1) * 128],
                                qT2[g][:, :, i0:i0 + IBS],
                                start=(g == 0), stop=(g == HG - 1),
                                perf_mode=DR,
                            )
                        flush()
                        # p~ = SP * exp(score): bias ln(SP) inside the exp
                        pdst = (p2[jt // 2][:, jt % 2, :] if jt < F2JT
                                else p_t[jt])
                        nc.scalar.activation(
                            out=pdst, in_=sps,
                            func=mybir.ActivationFunctionType.Exp,
                            scale=SCALE / (SQK * SQK),
                            bias=lnsp[:, 0:1],
                        )
                        if pos == 0:
                            nc.vector.tensor_copy(out=Sf, in_=pdst)
                        else:
                            nc.vector.tensor_add(Sf, Sf, pdst)
                        # PV runs behind the scores pipeline (last i-block:
                        # PV is done ht-major below instead)
                        if ib < IB - 1:
                            if jt >= F2JT:
                                if jt < JT - 1:
                                    emit_pv16(og, jt, start=(pos == 0))
                            elif jt % 2 == 1:
                                emit_pv8(og, jt // 2)
                    S16 = spool.tile([128, IBS], DT, tag="S16", name=f"S16{ib}")
                    nc.vector.tensor_copy(out=S16, in_=Sf)

                    if ib < IB - 1:
                        def finish(og=og, S16=S16, ib=ib, i0=i0):
                            nc.sync.dma_start(out=den_d[ib], in_=S16)
                            for ht in range(HT):
                                nc.tensor.matmul(
                                    og[ht],
                                    vv[JT - 1][:, ht * 128:(ht + 1) * 128],
                                    p_t[JT - 1],
                                    start=False, stop=True,
                                )
                                ot = opool.tile([128, IBS], DT, tag="ot",
                                                name=f"ot{i0}_{ht}")
                                # keep the Act queue free for the next
                                # i-block's exps -> evacuate on DVE only
                                nc.vector.tensor_copy(out=ot, in_=og[ht])
                                dma = (nc.scalar.dma_start if ht % 2
                                       else nc.sync.dma_start)
                                dma(out=outT_d[ht * 128:(ht + 1) * 128,
                                               i0:i0 + IBS],
                                    in_=ot)
                        pending.append(finish)
                    else:
                        # last i-block: ht-major PV so each accumulator is
                        # complete (and its output DMA in flight) as early as
                        # possible -- shrinks the end-of-kernel tail.
                        flush()
                        nc.sync.dma_start(out=den_d[ib], in_=S16)
                        for ht in range(HT):
                            for pos2, jt in enumerate(jts_order):
                                if jt < F2JT:
                                    if jt % 2 == 1:
                                        continue
                                    nc.tensor.matmul(
                                        og[ht],
                                        v2[jt // 2][:, :,
                                                    ht * 128:(ht + 1) * 128],
                                        p2[jt // 2],
                                        start=False, stop=False,
                                        perf_mode=DR,
                                    )
                                else:
                                    nc.tensor.matmul(
                                        og[ht],
                                        vv[jt][:, ht * 128:(ht + 1) * 128],
                                        p_t[jt],
                                        start=(pos2 == 0),
                                        stop=(jt == JT - 1),
                                    )
                            ot = opool.tile([128, IBS], DT, tag="ot",
                                            name=f"ot{i0}_{ht}")
                            if ht % 2 == 0:
                                nc.vector.tensor_copy(out=ot, in_=og[ht])
                                dma = nc.sync.dma_start
                            else:
                                nc.scalar.activation(
                                    out=ot, in_=og[ht],
                                    func=mybir.ActivationFunctionType.Copy)
                                dma = nc.scalar.dma_start
                            dma(out=outT_d[ht * 128:(ht + 1) * 128,
                                           i0:i0 + IBS],
                                in_=ot)
                flush()
    nc.compile()
    return nc


@lru_cache(maxsize=1)
def _cached_program():
    return build_program()


def _prep_in_maps(x, W_qkv, b_qkv):
    x = np.asarray(x, dtype=np.float32)
    W_qkv = np.asarray(W_qkv, dtype=np.float32)
    b_qkv = np.asarray(b_qkv, dtype=np.float32)
    w16 = W_qkv.astype(np.float16)
    bq = b_qkv[0:H].astype(np.float32).reshape(HT, 128).T    # [128, HT]
    bk = b_qkv[H:2 * H].astype(np.float32).reshape(HT, 128).T
    # activation computes fp8(SQK*psum + bias) -> bias must carry SQK
    bqk = np.ascontiguousarray(
        SQK * np.concatenate([bq, bk], axis=1))  # [128, 2*HT]

    in_maps = []
    for core in range(NCORES):
        b, kh = core // 2, core % 2
        xb = x[b]  # [N, C] f32
        if kh == 1:
            # Rotate so this core's key rows occupy rows [0, NK). Queries are
            # also rotated; the host rotates this core's outputs back.
            xb = np.concatenate([xb[NK:], xb[:NK]], axis=0)
        xT = np.ascontiguousarray(xb.T).astype(np.float16)
        in_maps.append({"xT": xT, "w": w16, "bqk": bqk})
    return in_maps


def _combine(results, b_qkv):
    bv = np.asarray(b_qkv, dtype=np.float32)[2 * H:3 * H]
    out = np.empty((B, N, C), dtype=np.float32)
    for b in range(B):
        o0 = results[2 * b]["outT"].astype(np.float32)   # [H, N]
        d0 = results[2 * b]["den"].astype(np.float32).sum(axis=1).reshape(N)
        o1 = results[2 * b + 1]["outT"].astype(np.float32)
        d1 = results[2 * b + 1]["den"].astype(np.float32).sum(axis=1).reshape(N)
        # core (2b+1) worked in rotated query order; rotate back
        o1 = np.concatenate([o1[:, NK:], o1[:, :NK]], axis=1)
        d1 = np.concatenate([d1[NK:], d1[:NK]])
        # o is (SP*SV)-scaled, den SP-scaled; v bias commutes out of softmax
        out[b] = (((o0 + o1) / (d0 + d1)) / SV).T + bv
    return out


def kernel(x, W_qkv, b_qkv):
    nc = _cached_program()
    in_maps = _prep_in_maps(x, W_qkv, b_qkv)
    res = run_bass_kernel_spmd(nc, in_maps, core_ids=list(range(NCORES)))
    return _combine(res.results, b_qkv)


# revision 5
# speedup vs baseline: 1.0923x; 1.0901x over previous
"""Fused single-head attention (QKV projection + softmax(QK^T)V) on 8 trn2 cores.

Problem (hardcoded): x [4, 4096, 768] f32, W_qkv [768, 2304] f32, b_qkv [2304] f32.
  qkv = x @ W_qkv + b_qkv ; q,k,v = split(qkv, 3)
  out = softmax(q k^T / sqrt(768)) v          -> [4, 4096, 768] f32

Sharding: batch (4) x key-halves (2) -> 8 cores. Each core gets its half
of one batch's rows (pre-transposed on host to xT [768, 2048] fp16),
projects q/k/v for those 2048 rows only, AllGathers the pair's q8 halves
(which lands in original query order for both cores, ~33us, hidden under
the k/v projection), and computes PARTIAL attention sums over its keys
for all 4096 queries. The host combines pair partials:
out = ((o0 + o1) / (d0 + d1)) / SV + bv.  (v's bias commutes out of the
softmax average because the weights sum to 1, so it is applied on the host.)

Mixed precision (rel-err budget 2e-2, measured ~1.8e-2):
  - projection matmuls fp16 (fp8 here costs ~3.5e-2 alone - too noisy).
  - scores q.k^T: fp8e4m3 DoubleRow (2x PE throughput). q8/k8 are written
    by the projection-evacuation activation directly into the DoubleRow
    interleaved layout [128, 2, n] with h = 256g + 128i + p, scaled by
    SQK=16 (exp scale folds 1/SQK^2 back out).
  - PV: the first F2JT=8 j-tiles (half the keys) run fp8 DoubleRow
    (p8 = 8p e4m3, v8 = 16v e4m3), the rest fp16. Both paths are scaled
    p~=8p, v~=16v so they accumulate consistently in the fp32 PSUM.
  - outputs outT/den fp16 (halves output DMA).

Structure notes:
  - warm-up matmuls cover the HAM clock-gate ramp + initial DMA.
  - x r-block DMAs are staggered between the W column-block DMAs so the
    projection never starves.
  - each i-block's PV for the last j-tile runs ht-by-ht with its PSUM
    evacuation + output DMA chasing each matmul (epilogue pattern), so the
    6 accumulator banks free progressively instead of all-at-once.
PSUM budget (8 banks): scores 2 + out accumulators 6.
"""

import math
from contextlib import ExitStack
from functools import lru_cache

import numpy as np

import concourse.bacc as bacc
import concourse.bass as bass
import concourse.tile as tile
from concourse import mybir
from concourse.bass_utils import run_bass_kernel_spmd

B, N, C = 4, 4096, 768
H = 768          # head dim (== C)
H3 = 3 * H
NCORES = 8
NK = N // 2      # keys per core
DT = mybir.dt.float16
FP8 = mybir.dt.float8e4
F32 = mybir.dt.float32
SCALE = 1.0 / math.sqrt(H)
SQK = 16.0       # fp8 scale for q and k (q8 = SQK*(q+bq))
SV = 16.0        # scale for v (both fp8 and fp16 paths)
SP = 8.0         # scale for p = exp(score) (both paths); exp bias = ln(SP)
DR = mybir.MatmulPerfMode.DoubleRow

CT = C // 128    # 6 contraction tiles (c)
HT = H // 128    # 6 head tiles (h)
HG = HT // 2     # 3 DoubleRow groups of 256 head dims
JT = NK // 128   # 16 key tiles (j) per core
F2JT = 4         # first F2JT j-tiles use fp8 PV (error/speed knob;
                 # 8 measured 2.11e-2 rel err on hw - over the 2e-2 gate;
                 # 4 measures 1.75e-2)
RB = 8           # r-blocks of 512 over the 4096 rows
RBS = N // RB    # 512
KRB = RB // 2    # r-blocks that contain this core's keys (first 4)
IB = 8           # i-blocks of 512 over all 4096 queries
IBS = N // IB    # 512


def build_program():
    nc = bacc.Bacc(
        "TRN2",
        target_bir_lowering=False,
        debug=False,
        enable_asserts=False,
        num_devices=NCORES,
    )
    xT_d = nc.dram_tensor("xT", [C, NK], DT, kind="ExternalInput").ap()
    w_d = nc.dram_tensor("w", [C, H3], DT, kind="ExternalInput").ap()
    bqk_d = nc.dram_tensor("bqk", [128, 2 * HT], F32, kind="ExternalInput").ap()
    outT_d = nc.dram_tensor("outT", [H, N], DT, kind="ExternalOutput").ap()
    # per-partition partial softmax denominators; host sums over axis 1
    den_d = nc.dram_tensor("den", [IB, 128, IBS], DT, kind="ExternalOutput").ap()
    # q8 halves exchanged between the two cores of a batch via AllGather
    # (~33us measured on hw for this size; hidden under the k/v projection):
    # each core projects q only for its own 2048 queries (local cols 0:NK,
    # which are exactly its original-order half); the gather concatenates
    # the pair in replica order = original query order for both cores.
    qstage_d = nc.dram_tensor("qstage", [HG, 128, 2, NK], FP8,
                              kind="Internal").ap()
    qgath_d = nc.dram_tensor("qgath", [2, HG, 128, 2, NK], FP8,
                             kind="Internal").ap()

    with tile.TileContext(nc) as tc:
        with ExitStack() as ctx:
            persist = ctx.enter_context(tc.tile_pool(name="persist", bufs=1))

            # fp8 q/k in DoubleRow-interleaved layout: tile g holds head dims
            # [256g, 256g+256) as [128 partitions(p), 2(i), cols] with
            # h = 256g + 128i + p.
            kT2 = [persist.tile([128, 2, NK], FP8, tag=f"kT2{t}", name=f"kT2{t}")
                   for t in range(HG)]
            qT2 = [persist.tile([128, 2, N], FP8, tag=f"qT2{t}", name=f"qT2{t}")
                   for t in range(HG)]
            # v: fp8 DoubleRow tiles for j-tiles < F2JT (key j = 256gj+128i+p),
            # fp16 tiles for the rest
            v2 = [persist.tile([128, 2, H], FP8, tag=f"v2{t}", name=f"v2{t}")
                  for t in range(F2JT // 2)]
            vv = {jt: persist.tile([128, H], DT, tag=f"v{jt}", name=f"v{jt}")
                  for jt in range(F2JT, JT)}
            bqk = persist.tile([128, 2 * HT], F32, tag="bqk")

            # ---- Phase 1: QKV projection ----
            with tc.tile_pool(name="wpool", bufs=1) as wpool, \
                 tc.tile_pool(name="xpool", bufs=1) as xpool, \
                 tc.tile_pool(name="pj", bufs=4, space="PSUM") as pj, \
                 tc.tile_pool(name="pv", bufs=2, space="PSUM") as pv:

                ws = [wpool.tile([128, H3], DT, tag=f"w{t}", name=f"w{t}")
                      for t in range(CT)]
                # x lives in SBUF for all of phase 1 as 6 half strips (only
                # local cols 0:NK are ever projected); few big DMAs on the
                # otherwise-idle Pool queue (the SP queue's ~650ns per-DMA
                # issue cost serializes small-DMA plans).
                xf = [xpool.tile([128, NK], DT, tag=f"xf{t}", name=f"xf{t}")
                      for t in range(CT)]
                qT2own = [wpool.tile([128, 2, NK], FP8, tag=f"qo{t}",
                                     name=f"qo{t}") for t in range(HG)]

                # warm-up operand memset goes first on the Pool queue (61ns
                # there) so the warm-up matmuls start immediately
                warm_l = wpool.tile([128, 128], DT, tag="warml", name="warml")
                nc.gpsimd.memset(warm_l, 0.0)
                # r-block-0 x pieces first, split across the idle Pool and
                # Act DMA queues so they land within the warm-up window.
                for ct in range(CT):
                    q = nc.gpsimd if ct % 2 == 0 else nc.scalar
                    q.dma_start(out=xf[ct][:, 0:RBS],
                                in_=xT_d[ct * 128:(ct + 1) * 128, 0:RBS])
                # W: q columns first (projected before k/v), then k, then v.
                for ct in range(CT):
                    nc.sync.dma_start(out=ws[ct][:, 0:H],
                                      in_=w_d[ct * 128:(ct + 1) * 128, 0:H])
                nc.sync.dma_start(out=bqk, in_=bqk_d)
                for ct in range(CT):
                    nc.gpsimd.dma_start(
                        out=xf[ct][:, RBS:2 * RBS],
                        in_=xT_d[ct * 128:(ct + 1) * 128, RBS:2 * RBS])
                for ct in range(CT):
                    nc.sync.dma_start(out=ws[ct][:, H:2 * H],
                                      in_=w_d[ct * 128:(ct + 1) * 128, H:2 * H])
                for ct in range(CT):
                    nc.gpsimd.dma_start(
                        out=xf[ct][:, 2 * RBS:NK],
                        in_=xT_d[ct * 128:(ct + 1) * 128, 2 * RBS:NK])
                for ct in range(CT):
                    nc.sync.dma_start(out=ws[ct][:, 2 * H:H3],
                                      in_=w_d[ct * 128:(ct + 1) * 128, 2 * H:H3])

                # PE warm-up: ~4.3us of junk matmuls (no DMA deps) so the
                # HAM clock-gate reaches full rate while the first x/W DMAs
                # are still in flight.
                for i in range(40):
                    wp = pj.tile([128, RBS], F32, tag="pj", name=f"warm{i}")
                    nc.tensor.matmul(wp[:, 0:128], warm_l, warm_l,
                                     start=True, stop=True)

                # q projection (own half only), then stage out + AllGather;
                # the cost model prices this collective at ~94us (40GB/s) but
                # it measures ~33us on hw, fully hidden under k/v below.
                for rb in range(KRB):
                    r0 = rb * RBS
                    for ht in range(HT):
                        ps = pj.tile([128, RBS], F32, tag="pj")
                        for ct in range(CT):
                            nc.tensor.matmul(
                                ps,
                                ws[ct][:, ht * 128:(ht + 1) * 128],
                                xf[ct][:, r0:r0 + RBS],
                                start=(ct == 0), stop=(ct == CT - 1),
                            )
                        # q8 = fp8(SQK*(q + b)) into interleaved slice
                        # (g, i) = (ht//2, ht%2); host pre-scales bias.
                        nc.scalar.activation(
                            out=qT2own[ht // 2][:, ht % 2, r0:r0 + RBS],
                            in_=ps,
                            func=mybir.ActivationFunctionType.Identity,
                            scale=SQK,
                            bias=bqk[:, ht:ht + 1],
                        )
                for g in range(HG):
                    nc.sync.dma_start(out=qstage_d[g], in_=qT2own[g])
                nc.gpsimd.collective_compute(
                    "AllGather", mybir.AluOpType.bypass,
                    replica_groups=[[0, 1], [2, 3], [4, 5], [6, 7]],
                    ins=[qstage_d], outs=[qgath_d],
                )
                for r in range(2):
                    for g in range(HG):
                        nc.gpsimd.dma_start(
                            out=qT2[g][:, :, r * NK:(r + 1) * NK],
                            in_=qgath_d[r, g])

                # k + v projection; overlaps the q exchange
                for rb in range(KRB):
                    r0 = rb * RBS
                    for ht in range(HT):
                        ps = pj.tile([128, RBS], F32, tag="pj")
                        for ct in range(CT):
                            nc.tensor.matmul(
                                ps,
                                ws[ct][:, H + ht * 128: H + (ht + 1) * 128],
                                xf[ct][:, r0:r0 + RBS],
                                start=(ct == 0), stop=(ct == CT - 1),
                            )
                        nc.scalar.activation(
                            out=kT2[ht // 2][:, ht % 2, r0:r0 + RBS],
                            in_=ps,
                            func=mybir.ActivationFunctionType.Identity,
                            scale=SQK,
                            bias=bqk[:, HT + ht:HT + ht + 1],
                        )

                    if True:
                        for j in range(RBS // 128):
                            jt = rb * (RBS // 128) + j
                            ps = pv.tile([128, H], F32, tag="pv")
                            for ct in range(CT):
                                xs = xf[ct][:, r0 + j * 128:r0 + (j + 1) * 128]
                                nc.tensor.matmul(
                                    ps[:, 0:512], xs, ws[ct][:, 2 * H: 2 * H + 512],
                                    start=(ct == 0), stop=(ct == CT - 1))
                                nc.tensor.matmul(
                                    ps[:, 512:H], xs, ws[ct][:, 2 * H + 512: 3 * H],
                                    start=(ct == 0), stop=(ct == CT - 1))
                            # v~ = SV*v (bias applied on host); ScalarE scale
                            # immediate (DVE tensor_scalar lowers to a slow
                            # TensorScalarPtr)
                            vdst = (v2[jt // 2][:, jt % 2, :] if jt < F2JT
                                    else vv[jt])
                            nc.scalar.activation(
                                out=vdst, in_=ps,
                                func=mybir.ActivationFunctionType.Identity,
                                scale=SV)

            # ---- Phase 2: attention (partial sums over this core's keys) ----
            with tc.tile_pool(name="ppool", bufs=1) as ppool, \
                 tc.tile_pool(name="opool", bufs=8) as opool, \
                 tc.tile_pool(name="spool", bufs=2) as spool, \
                 tc.tile_pool(name="ps_s", bufs=2, space="PSUM") as ps_s, \
                 tc.tile_pool(name="ps_o", bufs=6, space="PSUM") as ps_o:
                p2 = [ppool.tile([128, 2, IBS], FP8, tag=f"p2{t}", name=f"p2{t}")
                      for t in range(F2JT // 2)]
                p_t = {jt: ppool.tile([128, IBS], DT, tag=f"p{jt}", name=f"p{jt}")
                       for jt in range(F2JT, JT)}
                lnsp = ppool.tile([128, 1], F32, tag="lnsp", name="lnsp")
                nc.vector.memset(lnsp, math.log(SP))

                pending = []   # deferred work, flushed between PE groups

                def flush():
                    while pending:
                        pending.pop(0)()

                def emit_pv8(og, gj, start=False):
                    def go():
                        for ht in range(HT):
                            nc.tensor.matmul(
                                og[ht],
                                v2[gj][:, :, ht * 128:(ht + 1) * 128],
                                p2[gj],
                                start=start, stop=False,
                                perf_mode=DR,
                            )
                    pending.append(go)

                def emit_pv16(og, jt, start=False):
                    def go():
                        for ht in range(HT):
                            nc.tensor.matmul(
                                og[ht],
                                vv[jt][:, ht * 128:(ht + 1) * 128],
                                p_t[jt],
                                start=start, stop=False,
                            )
                    pending.append(go)

                # j-tile processing order: fp16 tiles first (their PV chases a
                # single exp, filling the pipeline early), the fp8 pairs
                # mid-sequence (their PV needs BOTH exps of the pair - burying
                # them among fp16 tiles hides that latency), fp16 again last;
                # the final j-tile finishes ht-by-ht with its PSUM evacuation
                # + output DMA chasing each matmul.
                jts_order = (list(range(F2JT, F2JT + 4)) + list(range(F2JT))
                             + list(range(F2JT + 4, JT)))
                for ib in range(IB):
                    i0 = ib * IBS
                    og = [ps_o.tile([128, IBS], F32, tag="o", name=f"o{ib}_{g}")
                          for g in range(HT)]
                    Sf = spool.tile([128, IBS], F32, tag="Sf", name=f"Sf{ib}")
                    for pos, jt in enumerate(jts_order):
                        sps = ps_s.tile([128, IBS], F32, tag="s")
                        for g in range(HG):
                            nc.tensor.matmul(
                                sps,
                                kT2[g][:, :, jt * 128:(jt + 1) * 128],
                                qT2[g][:, :, i0:i0 + IBS],
                                start=(g == 0), stop=(g == HG - 1),
                                perf_mode=DR,
                            )
                        flush()
                        # p~ = SP * exp(score): bias ln(SP) inside the exp
                        pdst = (p2[jt // 2][:, jt % 2, :] if jt < F2JT
                                else p_t[jt])
                        nc.scalar.activation(
                            out=pdst, in_=sps,
                            func=mybir.ActivationFunctionType.Exp,
                            scale=SCALE / (SQK * SQK),
                            bias=lnsp[:, 0:1],
                        )
                        if pos == 0:
                            nc.vector.tensor_copy(out=Sf, in_=pdst)
                        else:
                            nc.vector.tensor_add(Sf, Sf, pdst)
                        # PV runs behind the scores pipeline (last i-block:
                        # PV is done ht-major below instead)
                        if ib < IB - 1:
                            if jt >= F2JT:
                                if jt < JT - 1:
                                    emit_pv16(og, jt, start=(pos == 0))
                            elif jt % 2 == 1:
                                emit_pv8(og, jt // 2)
                    S16 = spool.tile([128, IBS], DT, tag="S16", name=f"S16{ib}")
                    nc.vector.tensor_copy(out=S16, in_=Sf)

                    if ib < IB - 1:
                        def finish(og=og, S16=S16, ib=ib, i0=i0):
                            nc.sync.dma_start(out=den_d[ib], in_=S16)
                            for ht in range(HT):
                                nc.tensor.matmul(
                                    og[ht],
                                    vv[JT - 1][:, ht * 128:(ht + 1) * 128],
                                    p_t[JT - 1],
                                    start=False, stop=True,
                                )
                                ot = opool.tile([128, IBS], DT, tag="ot",
                                                name=f"ot{i0}_{ht}")
                                # keep the Act queue free for the next
                                # i-block's exps -> evacuate on DVE only
                                nc.vector.tensor_copy(out=ot, in_=og[ht])
                                dma = (nc.scalar.dma_start if ht % 2
                                       else nc.sync.dma_start)
                                dma(out=outT_d[ht * 128:(ht + 1) * 128,
                                               i0:i0 + IBS],
                                    in_=ot)
                        pending.append(finish)
                    else:
                        # last i-block: ht-major PV so each accumulator is
                        # complete (and its output DMA in flight) as early as
                        # possible -- shrinks the end-of-kernel tail.
                        flush()
                        nc.sync.dma_start(out=den_d[ib], in_=S16)
                        for ht in range(HT):
                            for pos2, jt in enumerate(jts_order):
                                if jt < F2JT:
                                    if jt % 2 == 1:
                                        continue
                                    nc.tensor.matmul(
                                        og[ht],
                                        v2[jt // 2][:, :,
                                                    ht * 128:(ht + 1) * 128],
                                        p2[jt // 2],
                                        start=False, stop=False,
                                        perf_mode=DR,
                                    )
                                else:
                                    nc.tensor.matmul(
                                        og[ht],
                                        vv[jt][:, ht * 128:(ht + 1) * 128],
                                        p_t[jt],
                                        start=(pos2 == 0),
                                        stop=(jt == JT - 1),
                                    )
                            ot = opool.tile([128, IBS], DT, tag="ot",
                                            name=f"ot{i0}_{ht}")
                            if ht % 2 == 0:
                                nc.vector.tensor_copy(out=ot, in_=og[ht])
                                dma = nc.sync.dma_start
                            else:
                                nc.scalar.activation(
                                    out=ot, in_=og[ht],
                                    func=mybir.ActivationFunctionType.Copy)
                                dma = nc.scalar.dma_start
                            dma(out=outT_d[ht * 128:(ht + 1) * 128,
                                           i0:i0 + IBS],
                                in_=ot)
                flush()
    nc.compile()
    return nc


@lru_cache(maxsize=1)
def _cached_program():
    return build_program()


def _prep_in_maps(x, W_qkv, b_qkv):
    x = np.asarray(x, dtype=np.float32)
    W_qkv = np.asarray(W_qkv, dtype=np.float32)
    b_qkv = np.asarray(b_qkv, dtype=np.float32)
    w16 = W_qkv.astype(np.float16)
    bq = b_qkv[0:H].astype(np.float32).reshape(HT, 128).T    # [128, HT]
    bk = b_qkv[H:2 * H].astype(np.float32).reshape(HT, 128).T
    # activation computes fp8(SQK*psum + bias) -> bias must carry SQK
    bqk = np.ascontiguousarray(
        SQK * np.concatenate([bq, bk], axis=1))  # [128, 2*HT]

    in_maps = []
    for core in range(NCORES):
        b, kh = core // 2, core % 2
        # this core's half of the rows: its keys AND the queries it projects
        # (the q AllGather re-assembles original query order for both cores)
        xh = x[b][kh * NK:(kh + 1) * NK]      # [NK, C]
        xT = np.ascontiguousarray(xh.T).astype(np.float16)
        in_maps.append({"xT": xT, "w": w16, "bqk": bqk})
    return in_maps


def _combine(results, b_qkv):
    bv = np.asarray(b_qkv, dtype=np.float32)[2 * H:3 * H]
    out = np.empty((B, N, C), dtype=np.float32)
    for b in range(B):
        o0 = results[2 * b]["outT"].astype(np.float32)   # [H, N]
        d0 = results[2 * b]["den"].astype(np.float32).sum(axis=1).reshape(N)
        o1 = results[2 * b + 1]["outT"].astype(np.float32)
        d1 = results[2 * b + 1]["den"].astype(np.float32).sum(axis=1).reshape(N)
        # o is (SP*SV)-scaled, den SP-scaled; v bias commutes out of softmax
        out[b] = (((o0 + o1) / (d0 + d1)) / SV).T + bv
    return out


def kernel(x, W_qkv, b_qkv):
    nc = _cached_program()
    in_maps = _prep_in_maps(x, W_qkv, b_qkv)
    res = run_bass_kernel_spmd(nc, in_maps, core_ids=list(range(NCORES)))
    return _combine(res.results, b_qkv)
